# revision 2
# baseline (speedup 1.0000x reference)
"""DenseDilatedKnnGraph Trainium2 Bass kernel — hierarchical top-k rewrite.

k=16, dilation=2 KNN over L2-normalized points. Per core: one batch half
(4096 queries x 8192 candidates).

Scoring: S = 2*e - sq_m in ONE PE matmul with K=17 contraction
(candidate rows 0..15 = 2*xn, row 16 = -sq_m; query rows 0..15 = xn_q,
row 16 = ones). Ranking by S equals the reference's -dist ranking
(sq_n is a per-row constant) up to ~1ulp fused-rounding differences.

Candidates are normalized with the exact chain (DVE sum-squares, ACT sqrt,
DVE max eps, DVE IEEE reciprocal) because per-candidate scale error flips
rankings; queries use the cheap ACT-only chain (Square+accum, Rsqrt) since
per-query scale error is rank-neutral.

Top-32 per row, hierarchical (per 128-query block):
  stage1: 32x max8 over 256-wide chunks  -> V [128, 256] candidate values
  stage2: 4 rounds max8+match_replace on V -> top-32 values W, descending
  recovery: only even ranks 0,2,..,30 are output (dilation): compact the
  16 even-ranked values, 2x max_index over the full row -> indices.

Sharding: 8 cores; core c handles batch c//2, query half c%2.
"""
import sys
sys.path.insert(0, '/opt/trn_rl_repo')
import numpy as np

_CACHE = {}

B, C, N = 4, 16, 8192
QPC = N // 2          # queries per core
NBLK = QPC // 128     # 32 query blocks per core
CH = 256              # stage1 chunk width
NCH = N // CH         # 32 chunks
NEG = -1e30


def _build():
    import concourse.bass as bass
    import concourse.mybir as mybir
    import concourse.tile as tile
    from concourse import bacc
    from concourse.masks import make_identity

    F32 = mybir.dt.float32
    U32 = mybir.dt.uint32
    I32 = mybir.dt.int32
    AF = mybir.ActivationFunctionType

    nc = bacc.Bacc("TRN2", target_bir_lowering=False, debug=False, num_devices=8)

    xbT_d = nc.dram_tensor("xbT", [N, C], F32, kind="ExternalInput")
    xqT_d = nc.dram_tensor("xqT", [QPC, C], F32, kind="ExternalInput")
    qoff_d = nc.dram_tensor("qoff", [1, 1], I32, kind="ExternalInput")
    sc17_d = nc.dram_tensor("sc17", [17, 1], F32, kind="ExternalInput")
    ones_d = nc.dram_tensor("onesq", [1, QPC], F32, kind="ExternalInput")
    nn_o = nc.dram_tensor("nn_out", [QPC, 16], U32, kind="ExternalOutput")
    ctr_o = nc.dram_tensor("ctr_out", [QPC, 16], I32, kind="ExternalOutput")

    with tile.TileContext(nc) as tc:
        with tc.tile_pool(name="per", bufs=1) as per, \
             tc.tile_pool(name="nrm", bufs=3) as nrm, \
             tc.tile_pool(name="sco", bufs=2) as sco, \
             tc.tile_pool(name="chk", bufs=2) as chk, \
             tc.tile_pool(name="ps", bufs=3, space="PSUM") as ps, \
             tc.tile_pool(name="pst", bufs=2, space="PSUM") as pst:

            ident = per.tile([128, 128], F32)
            make_identity(nc, ident[:])

            xnT = per.tile([17, N], F32)     # rows 0..15: 2*xn cand, row 16: -sq_m
            wT = per.tile([17, QPC], F32)    # rows 0..15: xn_q, row 16: ones
            nc.sync.dma_start(wT[16:17, :], ones_d[:])
            # per-partition scales for the candidate psum->sbuf copy
            sc17 = per.tile([17, 1], F32)
            nc.sync.dma_start(sc17[:], sc17_d[:])

            # ---- center indices: one iota + add, one DMA out ----
            qb1 = per.tile([128, 1], I32)
            nc.sync.dma_start(qb1[:], qoff_d[:].to_broadcast((128, 1)))
            ctr_all = per.tile([128, NBLK * 16], I32)   # [p, i*16+k] = 128*i + p
            nc.gpsimd.iota(ctr_all[:], pattern=[[128, NBLK], [0, 16]],
                           base=0, channel_multiplier=1)
            nc.gpsimd.tensor_add(ctr_all[:], ctr_all[:],
                                 qb1[:].to_broadcast((128, NBLK * 16)))
            nc.sync.dma_start(
                ctr_o[:].rearrange("(i p) k -> p i k", i=NBLK, p=128),
                ctr_all[:])

            # ---- phases A (candidates -> xnT) and B (queries -> wT),
            # batched 4 point-tiles per group, B interleaved into A ----
            def phase_a_group(g):
                # exact normalize chain (per-candidate scale error flips ranks)
                xt4 = nrm.tile([128, 64], F32, tag="xt", name=f"xtA{g}")
                nc.sync.dma_start(
                    xt4[:].rearrange("p (u c) -> p u c", u=4),
                    xbT_d[512 * g:512 * (g + 1), :].rearrange(
                        "(u p) c -> p u c", u=4, p=128))
                xx4 = nrm.tile([128, 64], F32, tag="xx", name=f"xxA{g}")
                nc.vector.tensor_mul(xx4[:], xt4[:], xt4[:])
                ss4 = nrm.tile([128, 4], F32, tag="ss", name=f"ssA{g}")
                nc.vector.reduce_sum(ss4[:], xx4[:].rearrange(
                    "p (u c) -> p u c", u=4), axis=mybir.AxisListType.X)
                nrm4 = nrm.tile([128, 4], F32, tag="nrm", name=f"nrmA{g}")
                nc.scalar.activation(nrm4[:], ss4[:], AF.Sqrt)
                nc.vector.tensor_scalar_max(nrm4[:], nrm4[:], 1e-12)
                rcp4 = nrm.tile([128, 4], F32, tag="rcp", name=f"rcpA{g}")
                nc.vector.reciprocal(rcp4[:], nrm4[:])
                xnsq4 = nrm.tile([128, 68], F32, tag="xnsq", name=f"xnsqA{g}")
                xn_view = xnsq4[:].rearrange("p (u d) -> p u d", d=17)[:, :, 0:16]
                nc.vector.tensor_mul(xn_view, xt4[:].rearrange(
                    "p (u c) -> p u c", u=4), rcp4[:].to_broadcast((128, 4, 16)))
                pp4 = nrm.tile([128, 64], F32, tag="pp", name=f"ppA{g}")
                nc.vector.tensor_mul(pp4[:].rearrange("p (u c) -> p u c", u=4),
                                     xn_view, xn_view)
                nc.vector.reduce_sum(
                    xnsq4[:].rearrange("p (u d) -> p u d", d=17)[:, :, 16:17],
                    pp4[:].rearrange("p (u c) -> p u c", u=4),
                    axis=mybir.AxisListType.X)
                trs = pst.tile([17, 512], F32, tag="trs", name=f"trsA{g}")
                for u in range(4):
                    nc.tensor.transpose(trs[:, 128 * u:128 * (u + 1)],
                                        xnsq4[:, 17 * u:17 * u + 17], ident[:])
                # rows 0..15 scaled by 2, row 16 by -1 (exact)
                nc.scalar.activation(xnT[:, 512 * g:512 * (g + 1)], trs[:],
                                     AF.Identity, scale=sc17[:])

            def phase_b_group(g):
                # query normalize: per-row scale error is rank-neutral, so the
                # ACT Square-accum sum order does not matter
                xt4 = nrm.tile([128, 64], F32, tag="xt", name=f"xtB{g}")
                nc.sync.dma_start(
                    xt4[:].rearrange("p (u c) -> p u c", u=4),
                    xqT_d[512 * g:512 * (g + 1), :].rearrange(
                        "(u p) c -> p u c", u=4, p=128))
                xx4 = nrm.tile([128, 64], F32, tag="xx", name=f"xxB{g}")
                ss4 = nrm.tile([128, 4], F32, tag="ss", name=f"ssB{g}")
                for u in range(4):
                    nc.scalar.activation(xx4[:, 16 * u:16 * (u + 1)],
                                         xt4[:, 16 * u:16 * (u + 1)],
                                         AF.Square, accum_out=ss4[:, u:u + 1])
                nrm4 = nrm.tile([128, 4], F32, tag="nrm", name=f"nrmB{g}")
                nc.scalar.activation(nrm4[:], ss4[:], AF.Sqrt)
                rcp4 = nrm.tile([128, 4], F32, tag="rcp", name=f"rcpB{g}")
                nc.vector.reciprocal(rcp4[:], nrm4[:])
                xn4 = nrm.tile([128, 64], F32, tag="xnsq", name=f"xnB{g}")
                nc.vector.tensor_mul(
                    xn4[:].rearrange("p (u c) -> p u c", u=4),
                    xt4[:].rearrange("p (u c) -> p u c", u=4),
                    rcp4[:].to_broadcast((128, 4, 16)))
                trs = pst.tile([16, 512], F32, tag="trs", name=f"trsB{g}")
                for u in range(4):
                    nc.tensor.transpose(trs[:, 128 * u:128 * (u + 1)],
                                        xn4[:, 16 * u:16 * (u + 1)], ident[:])
                nc.scalar.copy(wT[0:16, 512 * g:512 * (g + 1)], trs[:])

            for g in range(N // 512):
                if g < QPC // 512:
                    phase_b_group(g)
                phase_a_group(g)

            # ---- phase C: scores + hierarchical top-32 ----
            for i in range(NBLK):
                S = sco.tile([128, N], F32, tag="S", name=f"S{i}")
                for j in range(N // 512):
                    pe = ps.tile([128, 512], F32, tag="pe", name=f"pe{i}_{j}")
                    nc.tensor.matmul(pe[:], wT[:, 128 * i:128 * (i + 1)],
                                     xnT[:, 512 * j:512 * (j + 1)],
                                     start=True, stop=True)
                    nc.scalar.copy(S[:, 512 * j:512 * (j + 1)], pe[:])

                # stage1: per-chunk top-8 values
                V = chk.tile([128, NCH * 8], F32, tag="V", name=f"V{i}")
                for c in range(NCH):
                    nc.vector.max(V[:, 8 * c:8 * c + 8],
                                  S[:, CH * c:CH * (c + 1)])

                # stage2: top-32 of V, descending
                W = chk.tile([128, 32], F32, tag="W", name=f"W{i}")
                for r in range(4):
                    nc.vector.max(W[:, 8 * r:8 * r + 8], V[:])
                    if r < 3:
                        nc.vector.match_replace(V[:], W[:, 8 * r:8 * r + 8],
                                                V[:], NEG)

                # recovery: even ranks only
                We = chk.tile([128, 16], F32, tag="We", name=f"We{i}")
                nc.vector.tensor_copy(We[:], W[:, 0:32:2])
                idx = chk.tile([128, 16], U32, tag="idx", name=f"idx{i}")
                nc.vector.max_index(idx[:, 0:8], We[:, 0:8], S[:])
                nc.vector.max_index(idx[:, 8:16], We[:, 8:16], S[:])
                nc.sync.dma_start(nn_o[128 * i:128 * (i + 1), :], idx[:])

    nc.compile()
    return nc


def _get_nc():
    if 'nc' not in _CACHE:
        _CACHE['nc'] = _build()
    return _CACHE['nc']


def kernel(x) -> np.ndarray:
    from concourse.bass_utils import run_bass_kernel_spmd

    x = np.asarray(x)
    assert x.shape == (B, C, N, 1) and x.dtype == np.float32
    xs = x[:, :, :, 0]  # (B, C, N)

    in_maps = []
    for c in range(8):
        b, h = c // 2, c % 2
        in_maps.append({
            "xbT": np.ascontiguousarray(xs[b].T),                       # (N, C)
            "xqT": np.ascontiguousarray(xs[b, :, h * QPC:(h + 1) * QPC].T),
            "qoff": np.array([[h * QPC]], np.int32),
            "sc17": np.array([[2.0]] * 16 + [[-1.0]], np.float32),
            "onesq": np.ones((1, QPC), np.float32),
        })

    nc = _get_nc()
    res = run_bass_kernel_spmd(nc, in_maps, list(range(8)))

    nn = np.empty((B, N, 16), np.int32)
    ctr = np.empty((B, N, 16), np.int32)
    for c in range(8):
        b, h = c // 2, c % 2
        sl = slice(h * QPC, (h + 1) * QPC)
        nn[b, sl] = res.results[c]["nn_out"].view(np.int32)
        ctr[b, sl] = res.results[c]["ctr_out"]
    return np.stack([nn, ctr], axis=0)  # (2, B, N, 16) int32


# revision 4
# speedup vs baseline: 1.0061x; 1.0061x over previous
"""DenseDilatedKnnGraph Trainium2 Bass kernel — hierarchical top-k rewrite.

k=16, dilation=2 KNN over L2-normalized points. Per core: one batch half
(4096 queries x 8192 candidates).

Scoring: S = 2*e - sq_m in ONE PE matmul with K=17 contraction
(candidate rows 0..15 = 2*xn, row 16 = -sq_m; query rows 0..15 = xn_q,
row 16 = ones). Ranking by S equals the reference's -dist ranking
(sq_n is a per-row constant) up to ~1ulp fused-rounding differences.

Candidates are normalized with the exact chain (DVE sum-squares, ACT sqrt,
DVE max eps, DVE IEEE reciprocal) because per-candidate scale error flips
rankings; queries use the cheap ACT-only chain (Square+accum, Rsqrt) since
per-query scale error is rank-neutral.

Top-32 per row, hierarchical (per 128-query block):
  stage1: 32x max8 over 256-wide chunks  -> V [128, 256] candidate values
  stage2: 4 rounds max8+match_replace on V -> top-32 values W, descending
  recovery: only even ranks 0,2,..,30 are output (dilation): compact the
  16 even-ranked values, 2x max_index over the full row -> indices.

Sharding: 8 cores; core c handles batch c//2, query half c%2.
"""
import sys
sys.path.insert(0, '/opt/trn_rl_repo')
import numpy as np

_CACHE = {}

B, C, N = 4, 16, 8192
QPC = N // 2          # queries per core
NBLK = QPC // 128     # 32 query blocks per core
CH = 256              # stage1 chunk width
NCH = N // CH         # 32 chunks
NEG = -1e30


def _build():
    import concourse.bass as bass
    import concourse.mybir as mybir
    import concourse.tile as tile
    from concourse import bacc
    from concourse.masks import make_identity

    F32 = mybir.dt.float32
    U32 = mybir.dt.uint32
    I32 = mybir.dt.int32
    AF = mybir.ActivationFunctionType

    nc = bacc.Bacc("TRN2", target_bir_lowering=False, debug=False, num_devices=8)

    xbT_d = nc.dram_tensor("xbT", [N, C], F32, kind="ExternalInput")
    xqT_d = nc.dram_tensor("xqT", [QPC, C], F32, kind="ExternalInput")
    qoff_d = nc.dram_tensor("qoff", [1, 1], I32, kind="ExternalInput")
    sc17_d = nc.dram_tensor("sc17", [17, 1], F32, kind="ExternalInput")
    ones_d = nc.dram_tensor("onesq", [1, QPC], F32, kind="ExternalInput")
    nn_o = nc.dram_tensor("nn_out", [QPC, 16], U32, kind="ExternalOutput")
    ctr_o = nc.dram_tensor("ctr_out", [QPC, 16], I32, kind="ExternalOutput")

    with tile.TileContext(nc) as tc:
        with tc.tile_pool(name="per", bufs=1) as per, \
             tc.tile_pool(name="nrm", bufs=6) as nrm, \
             tc.tile_pool(name="sco", bufs=2) as sco, \
             tc.tile_pool(name="chk", bufs=2) as chk, \
             tc.tile_pool(name="ps", bufs=4, space="PSUM") as ps, \
             tc.tile_pool(name="pst", bufs=3, space="PSUM") as pst:

            ident = per.tile([128, 128], F32)
            make_identity(nc, ident[:])

            xnT = per.tile([17, N], F32)     # rows 0..15: 2*xn cand, row 16: -sq_m
            wT = per.tile([17, QPC], F32)    # rows 0..15: xn_q, row 16: ones
            nc.sync.dma_start(wT[16:17, :], ones_d[:])
            # per-partition scales for the candidate psum->sbuf copy
            sc17 = per.tile([17, 1], F32)
            nc.sync.dma_start(sc17[:], sc17_d[:])

            # ---- center indices: one iota + add, one DMA out ----
            qb1 = per.tile([128, 1], I32)
            nc.sync.dma_start(qb1[:], qoff_d[:].to_broadcast((128, 1)))
            ctr_all = per.tile([128, NBLK * 16], I32)   # [p, i*16+k] = 128*i + p
            nc.gpsimd.iota(ctr_all[:], pattern=[[128, NBLK], [0, 16]],
                           base=0, channel_multiplier=1)
            nc.gpsimd.tensor_add(ctr_all[:], ctr_all[:],
                                 qb1[:].to_broadcast((128, NBLK * 16)))
            nc.sync.dma_start(
                ctr_o[:].rearrange("(i p) k -> p i k", i=NBLK, p=128),
                ctr_all[:])

            # ---- phases A (candidates -> xnT) and B (queries -> wT),
            # batched 4 point-tiles per group, B interleaved into A ----
            def phase_a_group(g):
                # exact normalize chain (per-candidate scale error flips ranks)
                xt4 = nrm.tile([128, 64], F32, tag="xt", name=f"xtA{g}")
                nc.sync.dma_start(
                    xt4[:].rearrange("p (u c) -> p u c", u=4),
                    xbT_d[512 * g:512 * (g + 1), :].rearrange(
                        "(u p) c -> p u c", u=4, p=128))
                xx4 = nrm.tile([128, 64], F32, tag="xx", name=f"xxA{g}")
                nc.vector.tensor_mul(xx4[:], xt4[:], xt4[:])
                ss4 = nrm.tile([128, 4], F32, tag="ss", name=f"ssA{g}")
                nc.vector.reduce_sum(ss4[:], xx4[:].rearrange(
                    "p (u c) -> p u c", u=4), axis=mybir.AxisListType.X)
                nrm4 = nrm.tile([128, 4], F32, tag="nrm", name=f"nrmA{g}")
                nc.scalar.activation(nrm4[:], ss4[:], AF.Sqrt)
                nc.vector.tensor_scalar_max(nrm4[:], nrm4[:], 1e-12)
                rcp4 = nrm.tile([128, 4], F32, tag="rcp", name=f"rcpA{g}")
                nc.vector.reciprocal(rcp4[:], nrm4[:])
                xnsq4 = nrm.tile([128, 68], F32, tag="xnsq", name=f"xnsqA{g}")
                xn_view = xnsq4[:].rearrange("p (u d) -> p u d", d=17)[:, :, 0:16]
                nc.vector.tensor_mul(xn_view, xt4[:].rearrange(
                    "p (u c) -> p u c", u=4), rcp4[:].to_broadcast((128, 4, 16)))
                pp4 = nrm.tile([128, 64], F32, tag="pp", name=f"ppA{g}")
                nc.vector.tensor_mul(pp4[:].rearrange("p (u c) -> p u c", u=4),
                                     xn_view, xn_view)
                nc.vector.reduce_sum(
                    xnsq4[:].rearrange("p (u d) -> p u d", d=17)[:, :, 16:17],
                    pp4[:].rearrange("p (u c) -> p u c", u=4),
                    axis=mybir.AxisListType.X)
                trs = pst.tile([17, 512], F32, tag="trs", name=f"trsA{g}")
                for u in range(4):
                    nc.tensor.transpose(trs[:, 128 * u:128 * (u + 1)],
                                        xnsq4[:, 17 * u:17 * u + 17], ident[:])
                # rows 0..15 scaled by 2, row 16 by -1 (exact)
                nc.scalar.activation(xnT[:, 512 * g:512 * (g + 1)], trs[:],
                                     AF.Identity, scale=sc17[:])

            def phase_b_group(g):
                # query normalize: per-row scale error is rank-neutral, so the
                # ACT Square-accum sum order does not matter
                xt4 = nrm.tile([128, 64], F32, tag="xt", name=f"xtB{g}")
                nc.sync.dma_start(
                    xt4[:].rearrange("p (u c) -> p u c", u=4),
                    xqT_d[512 * g:512 * (g + 1), :].rearrange(
                        "(u p) c -> p u c", u=4, p=128))
                xx4 = nrm.tile([128, 64], F32, tag="xx", name=f"xxB{g}")
                ss4 = nrm.tile([128, 4], F32, tag="ss", name=f"ssB{g}")
                for u in range(4):
                    nc.scalar.activation(xx4[:, 16 * u:16 * (u + 1)],
                                         xt4[:, 16 * u:16 * (u + 1)],
                                         AF.Square, accum_out=ss4[:, u:u + 1])
                nrm4 = nrm.tile([128, 4], F32, tag="nrm", name=f"nrmB{g}")
                nc.scalar.activation(nrm4[:], ss4[:], AF.Sqrt)
                rcp4 = nrm.tile([128, 4], F32, tag="rcp", name=f"rcpB{g}")
                nc.vector.reciprocal(rcp4[:], nrm4[:])
                xn4 = nrm.tile([128, 64], F32, tag="xnsq", name=f"xnB{g}")
                nc.vector.tensor_mul(
                    xn4[:].rearrange("p (u c) -> p u c", u=4),
                    xt4[:].rearrange("p (u c) -> p u c", u=4),
                    rcp4[:].to_broadcast((128, 4, 16)))
                trs = pst.tile([16, 512], F32, tag="trs", name=f"trsB{g}")
                for u in range(4):
                    nc.tensor.transpose(trs[:, 128 * u:128 * (u + 1)],
                                        xn4[:, 16 * u:16 * (u + 1)], ident[:])
                nc.scalar.copy(wT[0:16, 512 * g:512 * (g + 1)], trs[:])

            for g in range(N // 512):
                if g < QPC // 512:
                    phase_b_group(g)
                phase_a_group(g)

            # ---- phase C: scores + hierarchical top-32 ----
            for i in range(NBLK):
                S = sco.tile([128, N], F32, tag="S", name=f"S{i}")
                for j in range(N // 512):
                    pe = ps.tile([128, 512], F32, tag="pe", name=f"pe{i}_{j}")
                    nc.tensor.matmul(pe[:], wT[:, 128 * i:128 * (i + 1)],
                                     xnT[:, 512 * j:512 * (j + 1)],
                                     start=True, stop=True)
                    nc.scalar.copy(S[:, 512 * j:512 * (j + 1)], pe[:])

                # stage1: per-chunk top-8 values
                V = chk.tile([128, NCH * 8], F32, tag="V", name=f"V{i}")
                for c in range(NCH):
                    nc.vector.max(V[:, 8 * c:8 * c + 8],
                                  S[:, CH * c:CH * (c + 1)])

                # stage2: top-32 of V, descending
                W = chk.tile([128, 32], F32, tag="W", name=f"W{i}")
                for r in range(4):
                    nc.vector.max(W[:, 8 * r:8 * r + 8], V[:])
                    if r < 3:
                        nc.vector.match_replace(V[:], W[:, 8 * r:8 * r + 8],
                                                V[:], NEG)

                # recovery: even ranks only (strided views of W)
                idx = chk.tile([128, 16], U32, tag="idx", name=f"idx{i}")
                nc.vector.max_index(idx[:, 0:8], W[:, 0:16:2], S[:])
                nc.vector.max_index(idx[:, 8:16], W[:, 16:32:2], S[:])
                nc.sync.dma_start(nn_o[128 * i:128 * (i + 1), :], idx[:])

    nc.compile()
    return nc


def _get_nc():
    if 'nc' not in _CACHE:
        _CACHE['nc'] = _build()
    return _CACHE['nc']


def kernel(x) -> np.ndarray:
    from concourse.bass_utils import run_bass_kernel_spmd

    x = np.asarray(x)
    assert x.shape == (B, C, N, 1) and x.dtype == np.float32
    xs = x[:, :, :, 0]  # (B, C, N)

    in_maps = []
    for c in range(8):
        b, h = c // 2, c % 2
        in_maps.append({
            "xbT": np.ascontiguousarray(xs[b].T),                       # (N, C)
            "xqT": np.ascontiguousarray(xs[b, :, h * QPC:(h + 1) * QPC].T),
            "qoff": np.array([[h * QPC]], np.int32),
            "sc17": np.array([[2.0]] * 16 + [[-1.0]], np.float32),
            "onesq": np.ones((1, QPC), np.float32),
        })

    nc = _get_nc()
    res = run_bass_kernel_spmd(nc, in_maps, list(range(8)))

    nn = np.empty((B, N, 16), np.int32)
    ctr = np.empty((B, N, 16), np.int32)
    for c in range(8):
        b, h = c // 2, c % 2
        sl = slice(h * QPC, (h + 1) * QPC)
        nn[b, sl] = res.results[c]["nn_out"].view(np.int32)
        ctr[b, sl] = res.results[c]["ctr_out"]
    return np.stack([nn, ctr], axis=0)  # (2, B, N, 16) int32
